# revision 1
# baseline (speedup 1.0000x reference)
"""Multi-head attention forward (softmax(Q K^T / sqrt(d)) V) on 8 NeuronCores.

Shapes (hardcoded): Q/K/V [4, 16, 2048, 64] f32 -> 64 (b*h) independent heads,
8 heads per core (sharded on the flattened b*h axis). attn_mask is all-zeros
and unused by the module, so it is never transferred.

Per-core kernel (Bass/Tile), v2 -- engine-balanced softmax design:
  * Q and K are pre-transposed on the host during sharding (QT/KT
    [head, d, seq]), so qT/kT land in SBUF by straight DMA -- the v1
    on-chip PE-transpose + DVE-copy input pipeline is gone.
  * heads processed as 4 pairs (A, B) packed into SBUF partition halves so
    the d_k=64 contraction of S^T = K Q^T row-packs two concurrent PE
    matmuls (tile_position row groups 0-63 / 64-127), all in float32r
    (1 cycle/row vs 4 for fp32).
  * S^T tiles land in PSUM ([128, 1024] = both heads x 512 queries, double
    buffered).  The 33.5M-element exp is the single biggest cost; v1 ran it
    all on ScalarE (~255us busy).  v2 splits it: 10 of every 16 k-tiles use
    the exact ScalarE activation, 6 use a Schraudolph exp2 bit-trick on the
    otherwise-idle VectorE (one tensor_scalar: i32 = round(A*s + B) writes
    the IEEE-754 bits of ~exp(s/8); ~1.8% RMS per-element, ~1.1e-2 final
    output rel err incl. softmax normalization).  ScalarE, VectorE and PE
    each land at ~10-11us per (pair, q-block): balanced pipeline.
  * O'^T = [V | 1]^T @ P accumulates in PSUM over the 16 k-tiles; the ones
    column makes the softmax row-sum ride along as output row 64.  V is
    DMA'd f32 and bitcast to float32r at the matmul (no cast copy).  mm2
    emission lags two activations so the next q-block's mm1 jumps ahead in
    PE priority order (keeps the exp engines dense across transitions).
  * Normalization: DVE copy -> PE transpose [65,128]->[128,65] -> DVE
    reciprocal + broadcast tensor-tensor multiply -> DMA out, spread in
    small steps across the next block's iterations.
"""

import numpy as np

import concourse.bacc as bacc
import concourse.bass as bass
import concourse.mybir as mybir
import concourse.tile as tile
from concourse.bass_utils import run_bass_kernel_spmd
from concourse.masks import make_identity

B, H, SEQ, DK = 4, 16, 2048, 64
N_CORES = 8
HPC = (B * H) // N_CORES  # heads per core = 8
N_PAIRS = HPC // 2
SCALE = 1.0 / np.sqrt(DK)  # 0.125
P = 128
QB = 512  # q-block width (one PSUM bank of f32)
N_QB = SEQ // QB
N_KT = SEQ // P  # 16 k-tiles
F32 = mybir.dt.float32
F32R = mybir.dt.float32r
I16 = mybir.dt.int16
BF16 = mybir.dt.bfloat16
EXP = mybir.ActivationFunctionType.Exp

# Schraudolph exp2 bit trick on bf16: int16 bits(exp(s*SCALE)) ~=
# round(EXP_A*s + EXP_B).  EXP_C tuned for min RMS relative error (~1.78%)
# over scores ~ N(0, 8); end-to-end output rel err ~1.1e-2.
EXP_C = 7.4
EXP_A = float(2.0**7 * np.log2(np.e) * SCALE)
EXP_B = float(127.0 * 2.0**7 - EXP_C)
# k-tiles handled by VectorE (rest on ScalarE); interleaved so both exp
# engines run concurrently and mm2 consumes tiles roughly in order.
# (DVE ops drain-serialize on HW at ~1.5-1.8x their busy time, so DVE
# takes fewer tiles than ScalarE.)
DVE_KTS = (2, 4, 7, 9, 12, 14)


def build_attention_nc(repeat: int = 1) -> bass.Bass:
    nc = bacc.Bacc()
    QT = nc.dram_tensor("QT", [HPC, DK, SEQ], F32R, kind="ExternalInput")
    KT = nc.dram_tensor("KT", [HPC, DK, SEQ], F32R, kind="ExternalInput")
    V = nc.dram_tensor("V", [HPC, SEQ, DK], BF16, kind="ExternalInput")
    O = nc.dram_tensor("O", [HPC, SEQ, DK], F32, kind="ExternalOutput")

    import contextlib

    with tile.TileContext(nc) as tc:
        with (
            tc.tile_pool(name="consts", bufs=1) as consts,
            tc.tile_pool(name="inp", bufs=N_PAIRS) as inp,
            tc.tile_pool(name="pexp", bufs=6) as pexp,
            tc.tile_pool(name="onorm", bufs=2) as onorm,
            tc.tile_pool(name="psum_s", bufs=2, space="PSUM") as psum_s,
            tc.tile_pool(name="psum_o", bufs=1, space="PSUM") as psum_o,
            tc.tile_pool(name="psum_t", bufs=2, space="PSUM") as psum_t,
        ):
            ident = consts.tile([P, P], F32)
            make_identity(nc, ident)
            # tiny dummy exp: forces the ACT table load to happen during the
            # initial DMA ramp instead of blocking the first real activation
            warm = consts.tile([1, 1], F32)
            nc.gpsimd.memset(warm[:], 0.0)
            nc.scalar.activation(warm[:], warm[:], EXP)

            rep_ctx = (
                tc.For_i(0, repeat, 1) if repeat > 1 else contextlib.nullcontext()
            )
            with rep_ctx:
                _attention_body(nc, tc, QT, KT, V, O, ident, inp, pexp, onorm,
                                psum_s, psum_o, psum_t)
    return nc


def _emit_input_dmas(nc, QT, KT, V, inp):
    """Emit all input DMAs up front (HWDGE rings are FIFO; emission order ==
    consumption order).  Pair 0's first chunks are split fine so the first
    mm1/exp can start ~3us in; later pairs load as whole matrices."""
    handles = []
    tiles = []
    for pair in range(N_PAIRS):
        qT = inp.tile([P, SEQ], F32R, tag="qT", name=f"qT{pair}")
        kT = inp.tile([P, SEQ], F32R, tag="kT", name=f"kT{pair}")
        vts = [
            inp.tile([P, N_KT * (DK + 1)], BF16, tag=f"v{i}",
                     name=f"v{pair}_{i}")
            for i in range(2)
        ]
        tiles.append((qT, kT, vts))
        handles.append((
            2 * pair, 2 * pair + 1, qT, kT,
            vts[0].rearrange("p (n c) -> p n c", c=DK + 1),
            vts[1].rearrange("p (n c) -> p n c", c=DK + 1),
        ))

    def load_halves(dst, src_t, hA, hB, cols, eng=None):
        for ih, hh in ((0, hA), (1, hB)):
            nc.sync.dma_start(
                out=dst[ih * DK : (ih + 1) * DK, cols],
                in_=src_t[hh][:, cols],
            )

    def load_v(pair, hA, hB):
        vts = tiles[pair][2]
        for i, hh in ((0, hA), (1, hB)):
            vv = vts[i].rearrange("p (n c) -> p n c", c=DK + 1)
            nc.gpsimd.memset(vv[:, :, DK : DK + 1], 1.0)
            nc.sync.dma_start(
                out=vv[:, :, 0:DK],
                in_=V[hh].rearrange("(n p) d -> p n d", p=P),
            )

    # pair 0 is ramp-critical: k chunk 0 and q chunk 0 first (the scalar
    # HWDGE ring is busy with the ACT warm-up table load, so everything
    # rides the sync ring), then the rest of K (mm1's kt loop needs all of
    # kT before the first q-block finishes), then V, then the remaining q
    # chunks.
    qT0, kT0, _ = tiles[0]
    load_halves(kT0, KT, 0, 1, slice(0, QB), nc.sync)
    load_halves(qT0, QT, 0, 1, slice(0, QB), nc.sync)
    for c in range(1, 4):
        load_halves(kT0, KT, 0, 1, slice(c * QB, (c + 1) * QB), nc.sync)
    load_v(0, 0, 1)
    for c in range(1, 4):
        load_halves(qT0, QT, 0, 1, slice(c * QB, (c + 1) * QB), nc.sync)
    for pair in range(1, N_PAIRS):
        hA, hB = 2 * pair, 2 * pair + 1
        qT, kT, _ = tiles[pair]
        load_halves(kT, KT, hA, hB, slice(0, SEQ), nc.sync)
        load_halves(qT, QT, hA, hB, slice(0, SEQ), nc.sync)
        load_v(pair, hA, hB)
    return handles


def _norm_steps(nc, O, ident, onorm, psum_t, o_ps, hA, hB, qb, last=False):
    """Generator emitting the normalization/output chain for one q-block in
    small steps (PE transposes one-at-a-time to avoid bursts)."""
    o_sb = onorm.tile([DK + 1, 2 * QB], F32, tag="osb", name=f"osb{hA}_{qb}")
    nc.vector.tensor_copy(o_sb[:], o_ps[:])
    yield
    for half, hh in ((0, hA), (1, hB)):
        t_ps = psum_t.tile([P, 4 * (DK + 1)], F32, tag="t", name=f"ot{hh}_{qb}")
        for t4 in range(4):
            col = half * QB + t4 * P
            nc.tensor.transpose(
                t_ps[:, t4 * (DK + 1) : (t4 + 1) * (DK + 1)],
                o_sb[:, col : col + P],
                ident[0 : DK + 1, 0 : DK + 1],
            )
            yield
        tv = t_ps.rearrange("p (t c) -> p t c", c=DK + 1)
        rc = onorm.tile([P, 4], F32, tag="rc", name=f"rc{hh}_{qb}")
        nc.vector.reciprocal(rc[:], tv[:, :, DK])
        out_sb = onorm.tile([P, 4 * DK], F32, tag="outsb", name=f"outsb{hh}_{qb}")
        nc.vector.tensor_tensor(
            out=out_sb.rearrange("p (t c) -> p t c", c=DK),
            in0=tv[:, :, 0:DK],
            in1=rc[:, :, None].broadcast_to([P, 4, DK]),
            op=mybir.AluOpType.mult,
        )
        yield
        # the kernel's final two output DMAs ride separate HWDGE rings
        # (ScalarE is idle at the tail) so they complete in parallel
        eng = nc.scalar if (last and half == 1) else nc.sync
        eng.dma_start(
            out=O[hh].rearrange("(m p) d -> p m d", p=P)[
                :, qb * 4 : (qb + 1) * 4, :
            ],
            in_=out_sb.rearrange("p (m d) -> p m d", d=DK),
        )
        yield


def _attention_body(nc, tc, QT, KT, V, O, ident, inp, pexp, onorm,
                    psum_s, psum_o, psum_t):
    handles = _emit_input_dmas(nc, QT, KT, V, inp)

    pend_mm2 = []  # deque of pending (vrs, o_ps, chunks, p_sbr), depth <= 2
    norm_ready = []  # norm args whose final mm2 has been emitted
    norm_wait = []  # norm args waiting on their final mm2
    norm_gen = None  # in-flight normalization generator

    def emit_mm2(keep=2):
        fired = False
        while len(pend_mm2) > keep:
            vrs, o_ps, chunks, p_sbr = pend_mm2.pop(0)
            for j, (kt, ih) in enumerate(chunks):
                nc.tensor.matmul(
                    o_ps[:, ih * QB : (ih + 1) * QB],
                    lhsT=vrs[ih][:, kt, :],
                    rhs=p_sbr[:, j * QB : (j + 1) * QB],
                    start=(kt == 0),
                    stop=(kt == N_KT - 1),
                )
            if chunks[-1][0] == N_KT - 1 and norm_wait:
                norm_ready.append(norm_wait.pop(0))
        return fired

    def drive_norm(n=1):
        nonlocal norm_gen
        if norm_gen is None and norm_ready:
            norm_gen = _norm_steps(nc, O, ident, onorm, psum_t,
                                   *norm_ready.pop(0))
        if norm_gen is None:
            return
        try:
            for _ in range(n):
                next(norm_gen)
        except StopIteration:
            norm_gen = None

    for pair in range(N_PAIRS):
        hA, hB, qTr, kTr, vAr, vBr = handles[pair]
        vrs = (vAr, vBr)
        for qb in range(N_QB):
            o_ps = psum_o.tile(
                [DK + 1, 2 * QB], F32, tag="o", name=f"ops{pair}_{qb}"
            )
            for kt in range(N_KT):
                s_ps = psum_s.tile(
                    [P, 2 * QB], F32, tag="s", name=f"sps{pair}_{qb}_{kt}"
                )
                # S^T for heads A (partitions 0:64) and B (64:128):
                # row-packed concurrent matmuls (contraction = d_k = 64)
                for ih in (0, 1):
                    nc.tensor.matmul(
                        s_ps[:, ih * QB : (ih + 1) * QB],
                        lhsT=kTr[ih * DK : (ih + 1) * DK, kt * P : (kt + 1) * P],
                        rhs=qTr[ih * DK : (ih + 1) * DK, qb * QB : (qb + 1) * QB],
                        start=True,
                        stop=True,
                    )
                p_sbr = pexp.tile(
                    [P, 2 * QB], BF16, tag="p", name=f"p{pair}_{qb}_{kt}"
                )
                if kt in DVE_KTS:
                    # Schraudolph: int16 bits of ~exp(SCALE*s) in one DVE op
                    nc.vector.tensor_scalar(
                        p_sbr.bitcast(I16)[:],
                        s_ps[:],
                        EXP_A,
                        EXP_B,
                        mybir.AluOpType.mult,
                        mybir.AluOpType.add,
                    )
                else:
                    nc.scalar.activation(p_sbr[:], s_ps[:], EXP,
                                         scale=float(SCALE))
                emit_mm2(keep=2)
                drive_norm()
                pend_mm2.append((vrs, o_ps, [(kt, 0), (kt, 1)], p_sbr))
            norm_wait.append(
                (o_ps, hA, hB, qb,
                 pair == N_PAIRS - 1 and qb == N_QB - 1)
            )
            while norm_gen is not None or norm_ready:
                drive_norm(100)
                if norm_gen is None and not norm_ready:
                    break
    emit_mm2(keep=0)
    while norm_gen is not None or norm_ready or norm_wait:
        if norm_gen is None and not norm_ready and norm_wait:
            break
        drive_norm(100)
    assert not norm_wait and not norm_ready and norm_gen is None


_NC_CACHE = {}


def _get_nc():
    if "nc" not in _NC_CACHE:
        nc = build_attention_nc()
        if not nc.is_finalized():
            nc.finalize()
        _NC_CACHE["nc"] = nc
    return _NC_CACHE["nc"]


def prep_inputs(Q, K, V):
    """Host-side sharding/layout prep: transpose Q/K per head, cast V to
    bf16 (mm2 runs in bf16)."""
    import ml_dtypes

    QTf = np.ascontiguousarray(
        np.asarray(Q, dtype=np.float32).reshape(B * H, SEQ, DK).transpose(0, 2, 1)
    )
    KTf = np.ascontiguousarray(
        np.asarray(K, dtype=np.float32).reshape(B * H, SEQ, DK).transpose(0, 2, 1)
    )
    Vb = np.ascontiguousarray(
        np.asarray(V, dtype=np.float32).reshape(B * H, SEQ, DK)
    ).astype(ml_dtypes.bfloat16)
    return {"QT": QTf, "KT": KTf, "V": Vb}


def run(Q, K, V, trace=False):
    nc = _get_nc()
    full = prep_inputs(Q, K, V)
    in_maps = [
        {k: v[c * HPC : (c + 1) * HPC] for k, v in full.items()}
        for c in range(N_CORES)
    ]
    res = run_bass_kernel_spmd(nc, in_maps, list(range(N_CORES)), trace=trace)
    out = np.concatenate([r["O"] for r in res.results], axis=0)
    return out.reshape(B, H, SEQ, DK).astype(np.float32), res


def kernel(Q, K, V, attn_mask=None):
    out, _ = run(Q, K, V, trace=False)
    return out

